# revision 35
# baseline (speedup 1.0000x reference)
"""Pointer-generator attention kernel for 8 TRN2 NeuronCores.

Computation (per batch b):
    enc_feat = h[b] @ W_h.T                       # [T, N]
    att      = enc_feat + dec_fea[b] + cov[b,:,None] * W_c
    scores   = tanh(att) @ v                      # [T]
    attn     = exp(scores) * mask / sum(...)      # [T]
    c_t      = attn @ h[b]                        # [N]
    cov_new  = cov + attn

Sharding: data-parallel over batch, 8 batches per core, no collectives.

Device-side layout (per core):
    hT [8, N, T] in bf16 (cast on host) -- h transposed per batch, so the
    contraction dim n sits on SBUF partitions for the main matmul AND the
    t axis is the free dim for the pass-B reduce.  h is read from HBM once.
    att tiles [m=128, t=1024]: lhsT = W_hT chunk (stationary), rhs = hT.
    The rank-1 terms (dec_fea[m] x 1 + W_c[m] x cov[t]) are folded into the
    same PSUM accumulation group as one extra K=2 matmul.
    tanh on ScalarE (PSUM->SBUF eviction), v-dot as M=1 matmuls on PE,
    softmax on single-partition rows (exp has no overflow risk:
    |score| <= ||v||_1 ~ 26), pass B as fused multiply+reduce on VectorE
    over the resident hT tiles.  Matmuls in bf16, accumulation in fp32.
"""

import os
import sys

import numpy as np

sys.path.insert(0, "/opt/trn_rl_repo")

import concourse.bass as bass  # noqa: E402
import concourse.tile as tile  # noqa: E402
from concourse import mybir  # noqa: E402
from concourse.bass_utils import run_bass_kernel_spmd  # noqa: E402

B, T, N = 64, 1024, 1024
NCORES = 8
BL = B // NCORES  # 8 local batches per core
P = 128
KC = N // P  # 8 contraction chunks
MT = N // P  # 8 output row tiles
F32 = mybir.dt.float32
BF16 = mybir.dt.bfloat16
AF = mybir.ActivationFunctionType
ALU = mybir.AluOpType

LAST_EXEC_NS = None
_NC_CACHE = None


def build_bass():
    nc = bass.Bass()

    hT_h = nc.declare_dram_parameter("hT", [BL, N, T], BF16, isOutput=False)
    cov_h = nc.declare_dram_parameter("cov", [BL, T], F32, isOutput=False)
    covb_h = nc.declare_dram_parameter("covb", [BL, T], BF16, isOutput=False)
    mask_h = nc.declare_dram_parameter("mask", [BL, T], F32, isOutput=False)
    sT_h = nc.declare_dram_parameter("sT", [N, BL], BF16, isOutput=False)
    whT_h = nc.declare_dram_parameter("WhT", [N, N], BF16, isOutput=False)
    dwT_h = nc.declare_dram_parameter("decWT", [N, N], BF16, isOutput=False)
    decb_h = nc.declare_dram_parameter("decb", [1, N], BF16, isOutput=False)
    wcT_h = nc.declare_dram_parameter("WcT", [P, KC], F32, isOutput=False)
    vcol_h = nc.declare_dram_parameter("vcol", [P, KC], BF16, isOutput=False)

    ct_out = nc.declare_dram_parameter("out_ct", [BL, N], F32, isOutput=True)
    attn_out = nc.declare_dram_parameter("out_attn", [BL, T], F32, isOutput=True)
    cov_out = nc.declare_dram_parameter("out_cov", [BL, T], F32, isOutput=True)

    with tile.TileContext(nc) as tc:
        with (
            tc.tile_pool(name="const", bufs=1) as const,
            tc.tile_pool(name="ht", bufs=3) as htp,
            tc.tile_pool(name="att", bufs=3) as attp,
            tc.tile_pool(name="rows", bufs=2) as rowp,
            tc.tile_pool(name="rows1", bufs=1) as rowp1,
            tc.tile_pool(name="bc", bufs=BL) as bcp,
            tc.tile_pool(name="scr", bufs=2) as scrp,
            tc.tile_pool(name="psA", bufs=2, space="PSUM") as psA,
            tc.tile_pool(name="psS", bufs=2, space="PSUM") as psS,
        ):
            # ---- constants ----
            wh = const.tile([P, KC, N], BF16)  # [n%128, n//128, m]
            for kc in range(KC):
                nc.sync.dma_start(
                    out=wh[:, kc, :], in_=whT_h[kc * P : (kc + 1) * P, :]
                )
            vcol = const.tile([P, KC], BF16)
            nc.sync.dma_start(out=vcol[:], in_=vcol_h[:])
            ct_all = const.tile([P, BL, KC], F32)  # c_t[p + 128*kc] of batch b
            wcT = const.tile([P, KC], F32)  # W_c[mt*128+p] per-partition scalars
            nc.sync.dma_start(out=wcT[:], in_=wcT_h[:])
            covb_sb = const.tile([1, BL, T], BF16)  # cov rows (bf16)
            nc.sync.dma_start(out=covb_sb[:], in_=covb_h[:].unsqueeze(0))
            dec_feaT = const.tile([P, MT, BL], F32)  # dec_fea[m, b] bias layout
            ones_col = const.tile([1, P], BF16)  # lhsT for partition broadcast
            nc.any.memset(ones_col[:], 1.0)

            # ---- prologue: dec_fea = s_t_hat @ dec_W.T + dec_b  -> [b, m] ----
            with tc.tile_pool(name="prol", bufs=2) as prol:
                st = prol.tile([P, KC, BL], BF16, tag="st")
                nc.sync.dma_start(
                    out=st[:], in_=sT_h[:].rearrange("(kc p) b -> p kc b", p=P)
                )
                ones1 = prol.tile([1, BL], BF16, tag="ones1")
                nc.any.memset(ones1[:], 1.0)
                db = prol.tile([1, N], BF16, tag="db")
                nc.sync.dma_start(out=db[:], in_=decb_h[:])
                dwt = prol.tile([P, KC, N], BF16, tag="dwt")
                for kc in range(KC):
                    nc.sync.dma_start(
                        out=dwt[:, kc, :], in_=dwT_h[kc * P : (kc + 1) * P, :]
                    )
                # dec_feaT[m, b] = sum_n dec_W[m, n] s_t_hat[b, n] + dec_b[m]
                for mt in range(MT):
                    msl = slice(mt * P, (mt + 1) * P)
                    ps_d = psA.tile([P, BL], F32, tag="psA")
                    for kc in range(KC):
                        nc.tensor.matmul(
                            ps_d[:, :],
                            dwt[:, kc, msl],
                            st[:, kc, :],
                            start=(kc == 0),
                            stop=False,
                        )
                    nc.tensor.matmul(
                        ps_d[:, :], db[:, msl], ones1[:],
                        start=False, stop=True,
                    )
                    nc.vector.tensor_copy(dec_feaT[:, mt, :], ps_d[:, :])

            # ---- main loop over local batches ----
            def load_ht(b):
                t = htp.tile([P, KC, T], BF16, tag="ht")
                for kc in range(KC):
                    nc.sync.dma_start(
                        out=t[:, kc, :], in_=hT_h[b, kc * P : (kc + 1) * P, :]
                    )
                return t

            def load_rows(b):
                mrow = rowp.tile([1, T], F32, tag="mask")
                nc.sync.dma_start(out=mrow[:], in_=mask_h[b : b + 1, :])
                covrow = rowp.tile([1, T], F32, tag="covrow")
                nc.sync.dma_start(out=covrow[:], in_=cov_h[b : b + 1, :])
                return mrow, covrow

            ht_next = load_ht(0)
            rows_next = load_rows(0)
            for b in range(BL):
                ht = ht_next
                mrow, covrow = rows_next

                # cov[b] broadcast across partitions (for the W_c rank-1 term)
                cov_bc = bcp.tile([P, T], BF16, tag="covbc")
                ps_cb = psS.tile([P, T], F32, tag="psS")
                for th in range(2):
                    sl = slice(th * 512, (th + 1) * 512)
                    nc.tensor.matmul(
                        ps_cb[:, sl], ones_col[:], covb_sb[:, b, sl],
                        start=True, stop=True,
                    )
                nc.vector.tensor_copy(cov_bc[:], ps_cb[:])

                ps_sc = psS.tile([1, T], F32, tag="psS")
                for mt in range(MT):
                    msl = slice(mt * P, (mt + 1) * P)
                    ps_att = psA.tile([P, T], F32, tag="psA")
                    for th in range(2):
                        sl = slice(th * 512, (th + 1) * 512)
                        for kc in range(KC):
                            nc.tensor.matmul(
                                ps_att[:, sl],
                                wh[:, kc, msl],
                                ht[:, kc, sl],
                                start=(kc == 0),
                                stop=(kc == KC - 1),
                            )
                    # att += W_c[m] * cov[t]  (fused on DVE, in place on PSUM)
                    nc.vector.scalar_tensor_tensor(
                        out=ps_att[:, :], in0=cov_bc[:, :],
                        scalar=wcT[:, mt : mt + 1], in1=ps_att[:, :],
                        op0=ALU.mult, op1=ALU.add,
                    )
                    att = attp.tile([P, T], BF16, tag="att")
                    # att = tanh(psum + dec_fea[m])  (bias folds the dec term)
                    nc.scalar.activation(
                        att[:], ps_att[:], AF.Tanh,
                        bias=dec_feaT[:, mt, b : b + 1],
                    )
                    for th in range(2):
                        sl = slice(th * 512, (th + 1) * 512)
                        nc.tensor.matmul(
                            ps_sc[:, sl],
                            vcol[:, mt : mt + 1],
                            att[:, sl],
                            start=(mt == 0),
                            stop=(mt == MT - 1),
                        )

                # prefetch next batch while this batch's softmax runs
                if b + 1 < BL:
                    ht_next = load_ht(b + 1)
                    rows_next = load_rows(b + 1)

                # softmax over t (no max-subtraction: |score| <= ||v||_1 ~ 26)
                erow = rowp1.tile([1, T], F32, tag="erow")
                nc.scalar.activation(erow[:], ps_sc[:], AF.Exp)
                emrow = rowp1.tile([1, T], F32, tag="emrow")
                ssum = rowp1.tile([1, 1], F32, tag="ssum")
                nc.vector.tensor_mul(emrow[:], erow[:], mrow[:])
                nc.vector.tensor_reduce(
                    ssum[:], emrow[:], mybir.AxisListType.X, ALU.add
                )
                rinv = rowp1.tile([1, 1], F32, tag="rinv")
                nc.vector.reciprocal(rinv[:], ssum[:])
                arow = rowp.tile([1, T], F32, tag="arow")
                nc.vector.tensor_scalar_mul(arow[:], emrow[:], rinv[:])
                nc.sync.dma_start(out=attn_out[b : b + 1, :], in_=arow[:])
                cnrow = rowp1.tile([1, T], F32, tag="cnrow")
                nc.vector.tensor_add(cnrow[:], arow[:], covrow[:])
                nc.sync.dma_start(out=cov_out[b : b + 1, :], in_=cnrow[:])

                # pass B: c_t[n] = sum_t attn[t] * hT[n, t]
                abrow = rowp.tile([1, T], BF16, tag="abrow")
                nc.vector.tensor_copy(abrow[:], arow[:])
                abc = bcp.tile([P, T], BF16, tag="abc")
                ps_bc = psS.tile([P, T], F32, tag="psS")
                for th in range(2):
                    sl = slice(th * 512, (th + 1) * 512)
                    nc.tensor.matmul(
                        ps_bc[:, sl], ones_col[:], abrow[:, sl],
                        start=True, stop=True,
                    )
                nc.vector.tensor_copy(abc[:], ps_bc[:])
                for kc in range(KC):
                    sc = scrp.tile([P, T], BF16, tag="scr")
                    nc.vector.tensor_mul(sc[:], ht[:, kc, :], abc[:])
                    nc.vector.tensor_reduce(
                        ct_all[:, b, kc : kc + 1], sc[:],
                        mybir.AxisListType.X, ALU.add,
                    )
                # c_t[b] out: dest viewed [p, kc] (4B-strided, tiny, overlapped)
                nc.sync.dma_start(
                    out=ct_out[b : b + 1, :].rearrange("o (k p) -> (o p) k", p=P),
                    in_=ct_all[:, b, :],
                )

    _legalize_waits(nc)
    return nc


# Walrus rejects instructions whose sync-wait count exceeds the per-opcode
# descriptor slots ("Too many sync wait commands").  Tile can emit 2-3 waits
# on matmuls/DMAs at cross-engine convergence points.  Hoist surplus waits
# onto standalone InstEventSemaphore carriers inserted just before the
# offender in the same engine stream: the engine stalls on the carrier(s),
# then issues the real instruction with a single wait.  Engine streams are
# in-order, so this is semantics-preserving.
_WAIT_SKIP_OPS = {"InstEventSemaphore"}


def _legalize_waits(nc, limit=1):
    import bass_rust

    def make_carrier(engine, wait):
        return mybir.InstNoOp(
            name=nc.get_next_instruction_name(),
            text_hint="waitfix",
            bass_nofuse=True,
            engine=engine,
            sync_info=mybir.SyncInfo(on_wait=[wait], on_update=[]),
        )

    for fn in nc.m.functions:
        for blk in fn.blocks:
            il = blk.instructions
            i = 0
            while i < len(il):
                inst = il[i]
                op = type(inst).__name__
                si = getattr(inst, "sync_info", None)
                if (
                    op in _WAIT_SKIP_OPS
                    or si is None
                    or len(si.on_wait) <= limit
                ):
                    i += 1
                    continue
                waits = list(si.on_wait)
                keep, surplus = waits[-limit:], waits[:-limit]
                carriers = [make_carrier(inst.engine, w) for w in surplus]
                inst.sync_info = bass_rust.SyncInfo(
                    on_wait=keep, on_update=si.on_update
                )
                for k, ev in enumerate(carriers):
                    il.insert(i + k, ev)
                i += len(carriers) + 1


def _get_nc():
    global _NC_CACHE
    if _NC_CACHE is None:
        _NC_CACHE = build_bass()
    return _NC_CACHE


def kernel(s_t_hat, h, enc_padding_mask, coverage, W_h, W_c, dec_W, dec_b, v):
    global LAST_EXEC_NS
    import ml_dtypes

    bf16 = ml_dtypes.bfloat16
    s_t_hat = np.asarray(s_t_hat, dtype=np.float32)
    h = np.asarray(h, dtype=np.float32)
    enc_padding_mask = np.ascontiguousarray(
        np.asarray(enc_padding_mask, dtype=np.float32)
    )
    coverage = np.ascontiguousarray(np.asarray(coverage, dtype=np.float32))
    W_h = np.asarray(W_h, dtype=np.float32)
    W_c = np.asarray(W_c, dtype=np.float32).reshape(1, N)
    dec_W = np.asarray(dec_W, dtype=np.float32)
    dec_b = np.asarray(dec_b, dtype=np.float32).reshape(1, N)
    v = np.asarray(v, dtype=np.float32)

    hT = np.ascontiguousarray(np.transpose(h, (0, 2, 1)).astype(bf16))  # [B, N, T]
    WhT = np.ascontiguousarray(W_h.T.astype(bf16))  # [n, m]
    decWT = np.ascontiguousarray(dec_W.T.astype(bf16))  # [n, m]
    sT = np.ascontiguousarray(s_t_hat.T.astype(bf16))  # [n, B]
    vcol = np.ascontiguousarray(v.reshape(KC, P).T.astype(bf16))  # [p, kc]
    covb = coverage.astype(bf16)
    wcT = np.ascontiguousarray(
        W_c.reshape(KC, P).T.astype(np.float32)
    )  # [p, kc]
    decb_b = np.ascontiguousarray(dec_b.astype(bf16))

    in_maps = []
    for c in range(NCORES):
        bs = slice(c * BL, (c + 1) * BL)
        in_maps.append(
            {
                "hT": hT[bs],
                "cov": coverage[bs],
                "covb": covb[bs],
                "mask": enc_padding_mask[bs],
                "sT": np.ascontiguousarray(sT[:, bs]),
                "WhT": WhT,
                "decWT": decWT,
                "decb": decb_b,
                "WcT": wcT,
                "vcol": vcol,
            }
        )

    nc = _get_nc()
    trace = os.environ.get("BASS_KERNEL_TRACE", "0") == "1"
    res = run_bass_kernel_spmd(
        nc, in_maps, core_ids=list(range(NCORES)), trace=trace
    )
    LAST_EXEC_NS = res.exec_time_ns

    c_t = np.concatenate([res.results[c]["out_ct"] for c in range(NCORES)], axis=0)
    attn = np.concatenate(
        [res.results[c]["out_attn"] for c in range(NCORES)], axis=0
    )
    cov_new = np.concatenate(
        [res.results[c]["out_cov"] for c in range(NCORES)], axis=0
    )
    return (c_t, attn, cov_new)


# revision 38
# speedup vs baseline: 1.2516x; 1.2516x over previous
"""Pointer-generator attention kernel for 8 TRN2 NeuronCores.

Computation (per batch b):
    enc_feat = h[b] @ W_h.T                       # [T, N]
    att      = enc_feat + dec_fea[b] + cov[b,:,None] * W_c
    scores   = tanh(att) @ v                      # [T]
    attn     = exp(scores) * mask / sum(...)      # [T]
    c_t      = attn @ h[b]                        # [N]
    cov_new  = cov + attn

Sharding: data-parallel over batch, 8 batches per core, no collectives.

Device-side layout (per core):
    hT [8, N, T] in bf16 (cast on host) -- h transposed per batch, so the
    contraction dim n sits on SBUF partitions for the main matmul AND the
    t axis is the free dim for the pass-B reduce.  h is read from HBM once.
    att tiles [m=128, t=1024]: lhsT = W_hT chunk (stationary), rhs = hT.
    The rank-1 terms (dec_fea[m] x 1 + W_c[m] x cov[t]) are folded into the
    same PSUM accumulation group as one extra K=2 matmul.
    tanh on ScalarE (PSUM->SBUF eviction), v-dot as M=1 matmuls on PE,
    softmax on single-partition rows (exp has no overflow risk:
    |score| <= ||v||_1 ~ 26), pass B as fused multiply+reduce on VectorE
    over the resident hT tiles.  Matmuls in bf16, accumulation in fp32.
"""

import os
import sys

import numpy as np

sys.path.insert(0, "/opt/trn_rl_repo")

import concourse.bass as bass  # noqa: E402
import concourse.tile as tile  # noqa: E402
from concourse import mybir  # noqa: E402
from concourse.bass_utils import run_bass_kernel_spmd  # noqa: E402

B, T, N = 64, 1024, 1024
NCORES = 8
BL = B // NCORES  # 8 local batches per core
P = 128
KC = N // P  # 8 contraction chunks
MT = N // P  # 8 output row tiles
F32 = mybir.dt.float32
BF16 = mybir.dt.bfloat16
AF = mybir.ActivationFunctionType
ALU = mybir.AluOpType

LAST_EXEC_NS = None
_NC_CACHE = None


def build_bass():
    nc = bass.Bass()

    hT_h = nc.declare_dram_parameter("hT", [BL, N, T], BF16, isOutput=False)
    cov_h = nc.declare_dram_parameter("cov", [BL, T], F32, isOutput=False)
    covb_h = nc.declare_dram_parameter("covb", [BL, T], BF16, isOutput=False)
    mask_h = nc.declare_dram_parameter("mask", [BL, T], F32, isOutput=False)
    sT_h = nc.declare_dram_parameter("sT", [N, BL], BF16, isOutput=False)
    whT_h = nc.declare_dram_parameter("WhT", [N, N], BF16, isOutput=False)
    dwT_h = nc.declare_dram_parameter("decWT", [N, N], BF16, isOutput=False)
    decb_h = nc.declare_dram_parameter("decb", [1, N], BF16, isOutput=False)
    wcT_h = nc.declare_dram_parameter("WcT", [P, KC], F32, isOutput=False)
    vcol_h = nc.declare_dram_parameter("vcol", [P, KC], BF16, isOutput=False)

    ct_out = nc.declare_dram_parameter("out_ct", [BL, N], F32, isOutput=True)
    attn_out = nc.declare_dram_parameter("out_attn", [BL, T], F32, isOutput=True)
    cov_out = nc.declare_dram_parameter("out_cov", [BL, T], F32, isOutput=True)

    with tile.TileContext(nc) as tc:
        with (
            tc.tile_pool(name="const", bufs=1) as const,
            tc.tile_pool(name="ht", bufs=3) as htp,
            tc.tile_pool(name="att", bufs=3) as attp,
            tc.tile_pool(name="rows", bufs=2) as rowp,
            tc.tile_pool(name="rows1", bufs=1) as rowp1,
            tc.tile_pool(name="bc", bufs=BL) as bcp,
            tc.tile_pool(name="scr", bufs=2) as scrp,
            tc.tile_pool(name="psA", bufs=2, space="PSUM") as psA,
            tc.tile_pool(name="psS", bufs=2, space="PSUM") as psS,
        ):
            # ---- constants ----
            wh = const.tile([P, KC, N], BF16)  # [n%128, n//128, m]
            for kc in range(KC):
                nc.sync.dma_start(
                    out=wh[:, kc, :], in_=whT_h[kc * P : (kc + 1) * P, :]
                )
            vcol = const.tile([P, KC], BF16)
            nc.sync.dma_start(out=vcol[:], in_=vcol_h[:])
            ct_all = const.tile([P, BL, KC], F32)  # c_t[p + 128*kc] of batch b
            wcT = const.tile([P, KC], F32)  # W_c[mt*128+p] per-partition scalars
            nc.sync.dma_start(out=wcT[:], in_=wcT_h[:])
            covb_sb = const.tile([1, BL, T], BF16)  # cov rows (bf16)
            nc.sync.dma_start(out=covb_sb[:], in_=covb_h[:].unsqueeze(0))
            dec_feaT = const.tile([P, MT, BL], F32)  # dec_fea[m, b] bias layout
            ones_col = const.tile([1, P], BF16)  # lhsT for partition broadcast
            nc.any.memset(ones_col[:], 1.0)

            # ---- prologue: dec_fea = s_t_hat @ dec_W.T + dec_b  -> [b, m] ----
            with tc.tile_pool(name="prol", bufs=2) as prol:
                st = prol.tile([P, KC, BL], BF16, tag="st")
                nc.sync.dma_start(
                    out=st[:], in_=sT_h[:].rearrange("(kc p) b -> p kc b", p=P)
                )
                ones1 = prol.tile([1, BL], BF16, tag="ones1")
                nc.any.memset(ones1[:], 1.0)
                db = prol.tile([1, N], BF16, tag="db")
                nc.sync.dma_start(out=db[:], in_=decb_h[:])
                dwt = prol.tile([P, KC, N], BF16, tag="dwt")
                for kc in range(KC):
                    nc.sync.dma_start(
                        out=dwt[:, kc, :], in_=dwT_h[kc * P : (kc + 1) * P, :]
                    )
                # dec_feaT[m, b] = sum_n dec_W[m, n] s_t_hat[b, n] + dec_b[m]
                for mt in range(MT):
                    msl = slice(mt * P, (mt + 1) * P)
                    ps_d = psA.tile([P, BL], F32, tag="psA")
                    for kc in range(KC):
                        nc.tensor.matmul(
                            ps_d[:, :],
                            dwt[:, kc, msl],
                            st[:, kc, :],
                            start=(kc == 0),
                            stop=False,
                        )
                    nc.tensor.matmul(
                        ps_d[:, :], db[:, msl], ones1[:],
                        start=False, stop=True,
                    )
                    nc.vector.tensor_copy(dec_feaT[:, mt, :], ps_d[:, :])

            # ---- main loop over local batches ----
            def load_ht(b):
                t = htp.tile([P, KC, T], BF16, tag="ht")
                for kc in range(KC):
                    nc.sync.dma_start(
                        out=t[:, kc, :], in_=hT_h[b, kc * P : (kc + 1) * P, :]
                    )
                return t

            def load_rows(b):
                mrow = rowp.tile([1, T], F32, tag="mask")
                nc.sync.dma_start(out=mrow[:], in_=mask_h[b : b + 1, :])
                covrow = rowp.tile([1, T], F32, tag="covrow")
                nc.sync.dma_start(out=covrow[:], in_=cov_h[b : b + 1, :])
                return mrow, covrow

            # pass-B work is deferred and trickled into the next batch's
            # matmul loop so the DVE never bursts >1 op between PSUM
            # evictions (which would stall the PE on PSUM slot reuse).
            pending_pass_b = []

            def issue_pass_b_one():
                if pending_pass_b:
                    pending_pass_b.pop(0)()

            ht_next = load_ht(0)
            rows_next = load_rows(0)
            for b in range(BL):
                ht = ht_next
                mrow, covrow = rows_next

                # cov[b] broadcast across partitions (for the W_c rank-1 term)
                cov_bc = bcp.tile([P, T], BF16, tag="covbc")
                ps_cb = psS.tile([P, T], F32, tag="psS")
                for th in range(2):
                    sl = slice(th * 512, (th + 1) * 512)
                    nc.tensor.matmul(
                        ps_cb[:, sl], ones_col[:], covb_sb[:, b, sl],
                        start=True, stop=True,
                    )
                nc.vector.tensor_copy(cov_bc[:], ps_cb[:])

                ps_sc = psS.tile([1, T], F32, tag="psS")
                for mt in range(MT):
                    msl = slice(mt * P, (mt + 1) * P)
                    ps_att = psA.tile([P, T], F32, tag="psA")
                    for th in range(2):
                        sl = slice(th * 512, (th + 1) * 512)
                        for kc in range(KC):
                            nc.tensor.matmul(
                                ps_att[:, sl],
                                wh[:, kc, msl],
                                ht[:, kc, sl],
                                start=(kc == 0),
                                stop=(kc == KC - 1),
                            )
                    # att += W_c[m] * cov[t]  (fused on DVE, in place on PSUM)
                    nc.vector.scalar_tensor_tensor(
                        out=ps_att[:, :], in0=cov_bc[:, :],
                        scalar=wcT[:, mt : mt + 1], in1=ps_att[:, :],
                        op0=ALU.mult, op1=ALU.add,
                    )
                    att = attp.tile([P, T], BF16, tag="att")
                    # att = tanh(psum + dec_fea[m])  (bias folds the dec term)
                    nc.scalar.activation(
                        att[:], ps_att[:], AF.Tanh,
                        bias=dec_feaT[:, mt, b : b + 1],
                    )
                    for th in range(2):
                        sl = slice(th * 512, (th + 1) * 512)
                        nc.tensor.matmul(
                            ps_sc[:, sl],
                            vcol[:, mt : mt + 1],
                            att[:, sl],
                            start=(mt == 0),
                            stop=(mt == MT - 1),
                        )
                    issue_pass_b_one()

                # prefetch next batch while this batch's softmax runs
                if b + 1 < BL:
                    ht_next = load_ht(b + 1)
                    rows_next = load_rows(b + 1)

                # softmax over t (no max-subtraction: |score| <= ||v||_1 ~ 26)
                erow = rowp1.tile([1, T], F32, tag="erow")
                nc.scalar.activation(erow[:], ps_sc[:], AF.Exp)
                emrow = rowp1.tile([1, T], F32, tag="emrow")
                ssum = rowp1.tile([1, 1], F32, tag="ssum")
                nc.vector.scalar_tensor_tensor(
                    out=emrow[:], in0=erow[:], scalar=1.0, in1=mrow[:],
                    op0=ALU.bypass, op1=ALU.mult, accum_out=ssum[:],
                )
                rinv = rowp1.tile([1, 1], F32, tag="rinv")
                nc.vector.reciprocal(rinv[:], ssum[:])
                arow = rowp.tile([1, T], F32, tag="arow")
                nc.vector.tensor_scalar_mul(arow[:], emrow[:], rinv[:])
                nc.sync.dma_start(out=attn_out[b : b + 1, :], in_=arow[:])
                cnrow = rowp1.tile([1, T], F32, tag="cnrow")
                nc.vector.tensor_add(cnrow[:], arow[:], covrow[:])
                nc.sync.dma_start(out=cov_out[b : b + 1, :], in_=cnrow[:])

                # pass B: c_t[n] = sum_t attn[t] * hT[n, t]
                abrow = rowp.tile([1, T], BF16, tag="abrow")
                nc.vector.tensor_copy(abrow[:], arow[:])
                abc = bcp.tile([P, T], BF16, tag="abc")
                ps_bc = psS.tile([P, T], F32, tag="psS")
                for th in range(2):
                    sl = slice(th * 512, (th + 1) * 512)
                    nc.tensor.matmul(
                        ps_bc[:, sl], ones_col[:], abrow[:, sl],
                        start=True, stop=True,
                    )
                nc.vector.tensor_copy(abc[:], ps_bc[:])
                def make_pass_b(ht_=ht, abc_=abc, b_=b):
                    def stt_one(kc):
                        def run():
                            sc = scrp.tile([P, T], BF16, tag="scr")
                            nc.vector.scalar_tensor_tensor(
                                out=sc[:], in0=ht_[:, kc, :], scalar=1.0,
                                in1=abc_[:], op0=ALU.bypass, op1=ALU.mult,
                                accum_out=ct_all[:, b_, kc : kc + 1],
                            )
                        return run

                    def ct_dma():
                        # c_t[b] out: dest viewed [p, kc] (4B-strided, tiny)
                        nc.sync.dma_start(
                            out=ct_out[b_ : b_ + 1, :].rearrange(
                                "o (k p) -> (o p) k", p=P
                            ),
                            in_=ct_all[:, b_, :],
                        )

                    return [stt_one(kc) for kc in range(KC)] + [ct_dma]

                pending_pass_b.extend(make_pass_b())
                issue_pass_b_one()

            while pending_pass_b:
                issue_pass_b_one()

    _legalize_waits(nc)
    return nc


# Walrus rejects instructions whose sync-wait count exceeds the per-opcode
# descriptor slots ("Too many sync wait commands").  Tile can emit 2-3 waits
# on matmuls/DMAs at cross-engine convergence points.  Hoist surplus waits
# onto standalone InstEventSemaphore carriers inserted just before the
# offender in the same engine stream: the engine stalls on the carrier(s),
# then issues the real instruction with a single wait.  Engine streams are
# in-order, so this is semantics-preserving.
_WAIT_SKIP_OPS = {"InstEventSemaphore"}


def _legalize_waits(nc, limit=1):
    import bass_rust

    def make_carrier(engine, wait):
        return mybir.InstNoOp(
            name=nc.get_next_instruction_name(),
            text_hint="waitfix",
            bass_nofuse=True,
            engine=engine,
            sync_info=mybir.SyncInfo(on_wait=[wait], on_update=[]),
        )

    for fn in nc.m.functions:
        for blk in fn.blocks:
            il = blk.instructions
            i = 0
            while i < len(il):
                inst = il[i]
                op = type(inst).__name__
                si = getattr(inst, "sync_info", None)
                if (
                    op in _WAIT_SKIP_OPS
                    or si is None
                    or len(si.on_wait) <= limit
                ):
                    i += 1
                    continue
                waits = list(si.on_wait)
                keep, surplus = waits[-limit:], waits[:-limit]
                carriers = [make_carrier(inst.engine, w) for w in surplus]
                inst.sync_info = bass_rust.SyncInfo(
                    on_wait=keep, on_update=si.on_update
                )
                for k, ev in enumerate(carriers):
                    il.insert(i + k, ev)
                i += len(carriers) + 1


def _get_nc():
    global _NC_CACHE
    if _NC_CACHE is None:
        _NC_CACHE = build_bass()
    return _NC_CACHE


def kernel(s_t_hat, h, enc_padding_mask, coverage, W_h, W_c, dec_W, dec_b, v):
    global LAST_EXEC_NS
    import ml_dtypes

    bf16 = ml_dtypes.bfloat16
    s_t_hat = np.asarray(s_t_hat, dtype=np.float32)
    h = np.asarray(h, dtype=np.float32)
    enc_padding_mask = np.ascontiguousarray(
        np.asarray(enc_padding_mask, dtype=np.float32)
    )
    coverage = np.ascontiguousarray(np.asarray(coverage, dtype=np.float32))
    W_h = np.asarray(W_h, dtype=np.float32)
    W_c = np.asarray(W_c, dtype=np.float32).reshape(1, N)
    dec_W = np.asarray(dec_W, dtype=np.float32)
    dec_b = np.asarray(dec_b, dtype=np.float32).reshape(1, N)
    v = np.asarray(v, dtype=np.float32)

    hT = np.ascontiguousarray(np.transpose(h, (0, 2, 1)).astype(bf16))  # [B, N, T]
    WhT = np.ascontiguousarray(W_h.T.astype(bf16))  # [n, m]
    decWT = np.ascontiguousarray(dec_W.T.astype(bf16))  # [n, m]
    sT = np.ascontiguousarray(s_t_hat.T.astype(bf16))  # [n, B]
    vcol = np.ascontiguousarray(v.reshape(KC, P).T.astype(bf16))  # [p, kc]
    covb = coverage.astype(bf16)
    wcT = np.ascontiguousarray(
        W_c.reshape(KC, P).T.astype(np.float32)
    )  # [p, kc]
    decb_b = np.ascontiguousarray(dec_b.astype(bf16))

    in_maps = []
    for c in range(NCORES):
        bs = slice(c * BL, (c + 1) * BL)
        in_maps.append(
            {
                "hT": hT[bs],
                "cov": coverage[bs],
                "covb": covb[bs],
                "mask": enc_padding_mask[bs],
                "sT": np.ascontiguousarray(sT[:, bs]),
                "WhT": WhT,
                "decWT": decWT,
                "decb": decb_b,
                "WcT": wcT,
                "vcol": vcol,
            }
        )

    nc = _get_nc()
    trace = os.environ.get("BASS_KERNEL_TRACE", "0") == "1"
    res = run_bass_kernel_spmd(
        nc, in_maps, core_ids=list(range(NCORES)), trace=trace
    )
    LAST_EXEC_NS = res.exec_time_ns

    c_t = np.concatenate([res.results[c]["out_ct"] for c in range(NCORES)], axis=0)
    attn = np.concatenate(
        [res.results[c]["out_attn"] for c in range(NCORES)], axis=0
    )
    cov_new = np.concatenate(
        [res.results[c]["out_cov"] for c in range(NCORES)], axis=0
    )
    return (c_t, attn, cov_new)


# revision 40
# speedup vs baseline: 1.4202x; 1.1347x over previous
"""Pointer-generator attention kernel for 8 TRN2 NeuronCores.

Computation (per batch b):
    enc_feat = h[b] @ W_h.T                       # [T, N]
    att      = enc_feat + dec_fea[b] + cov[b,:,None] * W_c
    scores   = tanh(att) @ v                      # [T]
    attn     = exp(scores) * mask / sum(...)      # [T]
    c_t      = attn @ h[b]                        # [N]
    cov_new  = cov + attn

Sharding: data-parallel over batch, 8 batches per core, no collectives.

Device-side layout (per core):
    hT [8, N, T] in bf16 (cast on host) -- h transposed per batch, so the
    contraction dim n sits on SBUF partitions for the main matmul AND the
    t axis is the free dim for the pass-B reduce.  h is read from HBM once.
    att tiles [m=128, t=1024]: lhsT = W_hT chunk (stationary), rhs = hT.
    The rank-1 terms (dec_fea[m] x 1 + W_c[m] x cov[t]) are folded into the
    same PSUM accumulation group as one extra K=2 matmul.
    tanh on ScalarE (PSUM->SBUF eviction), v-dot as M=1 matmuls on PE,
    softmax on single-partition rows (exp has no overflow risk:
    |score| <= ||v||_1 ~ 26), pass B as fused multiply+reduce on VectorE
    over the resident hT tiles.  Matmuls in bf16, accumulation in fp32.
"""

import os
import sys

import numpy as np

sys.path.insert(0, "/opt/trn_rl_repo")

import concourse.bass as bass  # noqa: E402
import concourse.tile as tile  # noqa: E402
from concourse import mybir  # noqa: E402
from concourse.bass_utils import run_bass_kernel_spmd  # noqa: E402

B, T, N = 64, 1024, 1024
NCORES = 8
BL = B // NCORES  # 8 local batches per core
P = 128
KC = N // P  # 8 contraction chunks
MT = N // P  # 8 output row tiles
F32 = mybir.dt.float32
BF16 = mybir.dt.bfloat16
AF = mybir.ActivationFunctionType
ALU = mybir.AluOpType

LAST_EXEC_NS = None
_NC_CACHE = None


def build_bass():
    nc = bass.Bass()

    hT_h = nc.declare_dram_parameter("hT", [BL, N, T], BF16, isOutput=False)
    cov_h = nc.declare_dram_parameter("cov", [BL, T], F32, isOutput=False)
    covb_h = nc.declare_dram_parameter("covb", [BL, T], BF16, isOutput=False)
    mask_h = nc.declare_dram_parameter("mask", [BL, T], F32, isOutput=False)
    sT_h = nc.declare_dram_parameter("sT", [N, BL], BF16, isOutput=False)
    whT_h = nc.declare_dram_parameter("WhT", [N, N], BF16, isOutput=False)
    dwT_h = nc.declare_dram_parameter("decWT", [N, N], BF16, isOutput=False)
    decb_h = nc.declare_dram_parameter("decb", [1, N], BF16, isOutput=False)
    wcT_h = nc.declare_dram_parameter("WcT", [P, KC], F32, isOutput=False)
    vcol_h = nc.declare_dram_parameter("vcol", [P, KC], BF16, isOutput=False)

    ct_out = nc.declare_dram_parameter("out_ct", [BL, N], F32, isOutput=True)
    attn_out = nc.declare_dram_parameter("out_attn", [BL, T], F32, isOutput=True)
    cov_out = nc.declare_dram_parameter("out_cov", [BL, T], F32, isOutput=True)

    with tile.TileContext(nc) as tc:
        with (
            tc.tile_pool(name="const", bufs=1) as const,
            tc.tile_pool(name="ht", bufs=3) as htp,
            tc.tile_pool(name="att", bufs=3) as attp,
            tc.tile_pool(name="rows", bufs=2) as rowp,
            tc.tile_pool(name="rows1", bufs=1) as rowp1,
            tc.tile_pool(name="bc", bufs=BL) as bcp,
            tc.tile_pool(name="scr", bufs=2) as scrp,
            tc.tile_pool(name="psA", bufs=2, space="PSUM") as psA,
            tc.tile_pool(name="psS", bufs=1, space="PSUM") as psS,
            tc.tile_pool(name="psB", bufs=1, space="PSUM") as psB,
        ):
            # ---- constants ----
            wh = const.tile([P, KC, N], BF16)  # [n%128, n//128, m]
            for kc in range(KC):
                nc.sync.dma_start(
                    out=wh[:, kc, :], in_=whT_h[kc * P : (kc + 1) * P, :]
                )
            vcol = const.tile([P, KC], BF16)
            nc.sync.dma_start(out=vcol[:], in_=vcol_h[:])
            ct_all = const.tile([P, BL, KC], F32)  # c_t[p + 128*kc] of batch b
            wcT = const.tile([P, KC], F32)  # W_c[mt*128+p] per-partition scalars
            nc.sync.dma_start(out=wcT[:], in_=wcT_h[:])
            covb_sb = const.tile([1, BL, T], BF16)  # cov rows (bf16)
            nc.sync.dma_start(out=covb_sb[:], in_=covb_h[:].unsqueeze(0))
            dec_feaT = const.tile([P, MT, BL], F32)  # dec_fea[m, b] bias layout
            ones_col = const.tile([1, P], BF16)  # lhsT for partition broadcast
            nc.any.memset(ones_col[:], 1.0)

            cov_bc_all = []

            # ---- prologue: dec_fea = s_t_hat @ dec_W.T + dec_b  -> [b, m] ----
            with tc.tile_pool(name="prol", bufs=2) as prol:
                st = prol.tile([P, KC, BL], BF16, tag="st")
                nc.sync.dma_start(
                    out=st[:], in_=sT_h[:].rearrange("(kc p) b -> p kc b", p=P)
                )
                ones1 = prol.tile([1, BL], BF16, tag="ones1")
                nc.any.memset(ones1[:], 1.0)
                db = prol.tile([1, N], BF16, tag="db")
                nc.sync.dma_start(out=db[:], in_=decb_h[:])
                dwt = prol.tile([P, KC, N], BF16, tag="dwt")
                for kc in range(KC):
                    nc.sync.dma_start(
                        out=dwt[:, kc, :], in_=dwT_h[kc * P : (kc + 1) * P, :]
                    )
                # dec_feaT[m, b] = sum_n dec_W[m, n] s_t_hat[b, n] + dec_b[m]
                for mt in range(MT):
                    msl = slice(mt * P, (mt + 1) * P)
                    ps_d = psA.tile([P, BL], F32, tag="psA")
                    for kc in range(KC):
                        nc.tensor.matmul(
                            ps_d[:, :],
                            dwt[:, kc, msl],
                            st[:, kc, :],
                            start=(kc == 0),
                            stop=False,
                        )
                    nc.tensor.matmul(
                        ps_d[:, :], db[:, msl], ones1[:],
                        start=False, stop=True,
                    )
                    nc.vector.tensor_copy(dec_feaT[:, mt, :], ps_d[:, :])

                # broadcast every batch's cov row across partitions up front
                for b in range(BL):
                    cb = bcp.tile([P, T], BF16, tag="covbc")
                    ps_cb = psA.tile([P, T], F32, tag="psA")
                    for th in range(2):
                        sl = slice(th * 512, (th + 1) * 512)
                        nc.tensor.matmul(
                            ps_cb[:, sl], ones_col[:], covb_sb[:, b, sl],
                            start=True, stop=True,
                        )
                    nc.vector.tensor_copy(cb[:], ps_cb[:])
                    cov_bc_all.append(cb)

            # ---- main loop over local batches ----
            def load_ht(b):
                t = htp.tile([P, KC, T], BF16, tag="ht")
                for kc in range(KC):
                    nc.sync.dma_start(
                        out=t[:, kc, :], in_=hT_h[b, kc * P : (kc + 1) * P, :]
                    )
                return t

            def load_rows(b):
                mrow = rowp.tile([1, T], F32, tag="mask")
                nc.sync.dma_start(out=mrow[:], in_=mask_h[b : b + 1, :])
                covrow = rowp.tile([1, T], F32, tag="covrow")
                nc.sync.dma_start(out=covrow[:], in_=cov_h[b : b + 1, :])
                return mrow, covrow

            # pass-B work is deferred and trickled into the next batch's
            # matmul loop so the DVE never bursts >1 op between PSUM
            # evictions (which would stall the PE on PSUM slot reuse).
            pending_pass_b = []

            def issue_pass_b_one():
                if pending_pass_b:
                    pending_pass_b.pop(0)()

            ht_next = load_ht(0)
            rows_next = load_rows(0)
            for b in range(BL):
                ht = ht_next
                mrow, covrow = rows_next

                cov_bc = cov_bc_all[b]
                ps_sc = psS.tile([1, T], F32, tag="psS")
                for mt in range(MT):
                    msl = slice(mt * P, (mt + 1) * P)
                    ps_att = psA.tile([P, T], F32, tag="psA")
                    for th in range(2):
                        sl = slice(th * 512, (th + 1) * 512)
                        for kc in range(KC):
                            nc.tensor.matmul(
                                ps_att[:, sl],
                                wh[:, kc, msl],
                                ht[:, kc, sl],
                                start=(kc == 0),
                                stop=(kc == KC - 1),
                            )
                    # att += W_c[m] * cov[t]  (fused on DVE, in place on PSUM)
                    nc.vector.scalar_tensor_tensor(
                        out=ps_att[:, :], in0=cov_bc[:, :],
                        scalar=wcT[:, mt : mt + 1], in1=ps_att[:, :],
                        op0=ALU.mult, op1=ALU.add,
                    )
                    att = attp.tile([P, T], BF16, tag="att")
                    # att = tanh(psum + dec_fea[m])  (bias folds the dec term)
                    nc.scalar.activation(
                        att[:], ps_att[:], AF.Tanh,
                        bias=dec_feaT[:, mt, b : b + 1],
                    )
                    for th in range(2):
                        sl = slice(th * 512, (th + 1) * 512)
                        nc.tensor.matmul(
                            ps_sc[:, sl],
                            vcol[:, mt : mt + 1],
                            att[:, sl],
                            start=(mt == 0),
                            stop=(mt == MT - 1),
                        )
                    issue_pass_b_one()

                # prefetch next batch while this batch's softmax runs
                if b + 1 < BL:
                    ht_next = load_ht(b + 1)
                    rows_next = load_rows(b + 1)

                # softmax over t (no max-subtraction: |score| <= ||v||_1 ~ 26)
                erow = rowp1.tile([1, T], F32, tag="erow")
                nc.scalar.activation(erow[:], ps_sc[:], AF.Exp)
                emrow = rowp1.tile([1, T], F32, tag="emrow")
                ssum = rowp1.tile([1, 1], F32, tag="ssum")
                nc.vector.scalar_tensor_tensor(
                    out=emrow[:], in0=erow[:], scalar=1.0, in1=mrow[:],
                    op0=ALU.bypass, op1=ALU.mult, accum_out=ssum[:],
                )
                rinv = rowp1.tile([1, 1], F32, tag="rinv")
                nc.vector.reciprocal(rinv[:], ssum[:])
                arow = rowp.tile([1, T], F32, tag="arow")
                nc.vector.tensor_scalar_mul(arow[:], emrow[:], rinv[:])
                nc.sync.dma_start(out=attn_out[b : b + 1, :], in_=arow[:])
                cnrow = rowp1.tile([1, T], F32, tag="cnrow")
                nc.vector.tensor_add(cnrow[:], arow[:], covrow[:])
                nc.sync.dma_start(out=cov_out[b : b + 1, :], in_=cnrow[:])

                # pass B: c_t[n] = sum_t attn[t] * hT[n, t]
                abrow = rowp.tile([1, T], BF16, tag="abrow")
                nc.vector.tensor_copy(abrow[:], arow[:])
                abc = bcp.tile([P, T], BF16, tag="abc")

                def make_pass_b(ht_=ht, abc_=abc, b_=b, abrow_=abrow):
                    def bcast():
                        ps_bc = psB.tile([P, T], F32, tag="psB")
                        for th in range(2):
                            sl = slice(th * 512, (th + 1) * 512)
                            nc.tensor.matmul(
                                ps_bc[:, sl], ones_col[:], abrow_[:, sl],
                                start=True, stop=True,
                            )
                        nc.vector.tensor_copy(abc_[:], ps_bc[:])

                    def stt_one(kc):
                        def run():
                            sc = scrp.tile([P, T], BF16, tag="scr")
                            nc.vector.scalar_tensor_tensor(
                                out=sc[:], in0=ht_[:, kc, :], scalar=1.0,
                                in1=abc_[:], op0=ALU.bypass, op1=ALU.mult,
                                accum_out=ct_all[:, b_, kc : kc + 1],
                            )
                        return run

                    def ct_dma():
                        # c_t[b] out: dest viewed [p, kc] (4B-strided, tiny)
                        nc.sync.dma_start(
                            out=ct_out[b_ : b_ + 1, :].rearrange(
                                "o (k p) -> (o p) k", p=P
                            ),
                            in_=ct_all[:, b_, :],
                        )

                    return (
                        [bcast]
                        + [stt_one(kc) for kc in range(KC)]
                        + [ct_dma]
                    )

                pending_pass_b.extend(make_pass_b())
                issue_pass_b_one()
                issue_pass_b_one()

            while pending_pass_b:
                issue_pass_b_one()

    _legalize_waits(nc)
    return nc


# Walrus rejects instructions whose sync-wait count exceeds the per-opcode
# descriptor slots ("Too many sync wait commands").  Tile can emit 2-3 waits
# on matmuls/DMAs at cross-engine convergence points.  Hoist surplus waits
# onto standalone InstEventSemaphore carriers inserted just before the
# offender in the same engine stream: the engine stalls on the carrier(s),
# then issues the real instruction with a single wait.  Engine streams are
# in-order, so this is semantics-preserving.
_WAIT_SKIP_OPS = {"InstEventSemaphore"}


def _legalize_waits(nc, limit=1):
    import bass_rust

    def make_carrier(engine, wait):
        return mybir.InstNoOp(
            name=nc.get_next_instruction_name(),
            text_hint="waitfix",
            bass_nofuse=True,
            engine=engine,
            sync_info=mybir.SyncInfo(on_wait=[wait], on_update=[]),
        )

    for fn in nc.m.functions:
        for blk in fn.blocks:
            il = blk.instructions
            i = 0
            while i < len(il):
                inst = il[i]
                op = type(inst).__name__
                si = getattr(inst, "sync_info", None)
                if (
                    op in _WAIT_SKIP_OPS
                    or si is None
                    or len(si.on_wait) <= limit
                ):
                    i += 1
                    continue
                waits = list(si.on_wait)
                keep, surplus = waits[-limit:], waits[:-limit]
                carriers = [make_carrier(inst.engine, w) for w in surplus]
                inst.sync_info = bass_rust.SyncInfo(
                    on_wait=keep, on_update=si.on_update
                )
                for k, ev in enumerate(carriers):
                    il.insert(i + k, ev)
                i += len(carriers) + 1


def _get_nc():
    global _NC_CACHE
    if _NC_CACHE is None:
        _NC_CACHE = build_bass()
    return _NC_CACHE


def kernel(s_t_hat, h, enc_padding_mask, coverage, W_h, W_c, dec_W, dec_b, v):
    global LAST_EXEC_NS
    import ml_dtypes

    bf16 = ml_dtypes.bfloat16
    s_t_hat = np.asarray(s_t_hat, dtype=np.float32)
    h = np.asarray(h, dtype=np.float32)
    enc_padding_mask = np.ascontiguousarray(
        np.asarray(enc_padding_mask, dtype=np.float32)
    )
    coverage = np.ascontiguousarray(np.asarray(coverage, dtype=np.float32))
    W_h = np.asarray(W_h, dtype=np.float32)
    W_c = np.asarray(W_c, dtype=np.float32).reshape(1, N)
    dec_W = np.asarray(dec_W, dtype=np.float32)
    dec_b = np.asarray(dec_b, dtype=np.float32).reshape(1, N)
    v = np.asarray(v, dtype=np.float32)

    hT = np.ascontiguousarray(np.transpose(h, (0, 2, 1)).astype(bf16))  # [B, N, T]
    WhT = np.ascontiguousarray(W_h.T.astype(bf16))  # [n, m]
    decWT = np.ascontiguousarray(dec_W.T.astype(bf16))  # [n, m]
    sT = np.ascontiguousarray(s_t_hat.T.astype(bf16))  # [n, B]
    vcol = np.ascontiguousarray(v.reshape(KC, P).T.astype(bf16))  # [p, kc]
    covb = coverage.astype(bf16)
    wcT = np.ascontiguousarray(
        W_c.reshape(KC, P).T.astype(np.float32)
    )  # [p, kc]
    decb_b = np.ascontiguousarray(dec_b.astype(bf16))

    in_maps = []
    for c in range(NCORES):
        bs = slice(c * BL, (c + 1) * BL)
        in_maps.append(
            {
                "hT": hT[bs],
                "cov": coverage[bs],
                "covb": covb[bs],
                "mask": enc_padding_mask[bs],
                "sT": np.ascontiguousarray(sT[:, bs]),
                "WhT": WhT,
                "decWT": decWT,
                "decb": decb_b,
                "WcT": wcT,
                "vcol": vcol,
            }
        )

    nc = _get_nc()
    trace = os.environ.get("BASS_KERNEL_TRACE", "0") == "1"
    res = run_bass_kernel_spmd(
        nc, in_maps, core_ids=list(range(NCORES)), trace=trace
    )
    LAST_EXEC_NS = res.exec_time_ns

    c_t = np.concatenate([res.results[c]["out_ct"] for c in range(NCORES)], axis=0)
    attn = np.concatenate(
        [res.results[c]["out_attn"] for c in range(NCORES)], axis=0
    )
    cov_new = np.concatenate(
        [res.results[c]["out_cov"] for c in range(NCORES)], axis=0
    )
    return (c_t, attn, cov_new)


# revision 41
# speedup vs baseline: 1.4532x; 1.0232x over previous
"""Pointer-generator attention kernel for 8 TRN2 NeuronCores.

Computation (per batch b):
    enc_feat = h[b] @ W_h.T                       # [T, N]
    att      = enc_feat + dec_fea[b] + cov[b,:,None] * W_c
    scores   = tanh(att) @ v                      # [T]
    attn     = exp(scores) * mask / sum(...)      # [T]
    c_t      = attn @ h[b]                        # [N]
    cov_new  = cov + attn

Sharding: data-parallel over batch, 8 batches per core, no collectives.

Device-side layout (per core):
    hT [8, N, T] in bf16 (cast on host) -- h transposed per batch, so the
    contraction dim n sits on SBUF partitions for the main matmul AND the
    t axis is the free dim for the pass-B reduce.  h is read from HBM once.
    att tiles [m=128, t=1024]: lhsT = W_hT chunk (stationary), rhs = hT.
    The rank-1 terms (dec_fea[m] x 1 + W_c[m] x cov[t]) are folded into the
    same PSUM accumulation group as one extra K=2 matmul.
    tanh on ScalarE (PSUM->SBUF eviction), v-dot as M=1 matmuls on PE,
    softmax on single-partition rows (exp has no overflow risk:
    |score| <= ||v||_1 ~ 26), pass B as fused multiply+reduce on VectorE
    over the resident hT tiles.  Matmuls in bf16, accumulation in fp32.
"""

import os
import sys

import numpy as np

sys.path.insert(0, "/opt/trn_rl_repo")

import concourse.bass as bass  # noqa: E402
import concourse.tile as tile  # noqa: E402
from concourse import mybir  # noqa: E402
from concourse.bass_utils import run_bass_kernel_spmd  # noqa: E402

B, T, N = 64, 1024, 1024
NCORES = 8
BL = B // NCORES  # 8 local batches per core
P = 128
KC = N // P  # 8 contraction chunks
MT = N // P  # 8 output row tiles
F32 = mybir.dt.float32
BF16 = mybir.dt.bfloat16
AF = mybir.ActivationFunctionType
ALU = mybir.AluOpType

LAST_EXEC_NS = None
_NC_CACHE = None


def build_bass():
    nc = bass.Bass()

    hT_h = nc.declare_dram_parameter("hT", [BL, N, T], BF16, isOutput=False)
    cov_h = nc.declare_dram_parameter("cov", [BL, T], F32, isOutput=False)
    covb_h = nc.declare_dram_parameter("covb", [BL, T], BF16, isOutput=False)
    mask_h = nc.declare_dram_parameter("mask", [BL, T], F32, isOutput=False)
    sT_h = nc.declare_dram_parameter("sT", [N, BL], BF16, isOutput=False)
    whT_h = nc.declare_dram_parameter("WhT", [N, N], BF16, isOutput=False)
    dwT_h = nc.declare_dram_parameter("decWT", [N, N], BF16, isOutput=False)
    decb_h = nc.declare_dram_parameter("decb", [1, N], BF16, isOutput=False)
    wcT_h = nc.declare_dram_parameter("WcT", [P, KC], F32, isOutput=False)
    vcol_h = nc.declare_dram_parameter("vcol", [P, KC], BF16, isOutput=False)

    ct_out = nc.declare_dram_parameter("out_ct", [BL, N], F32, isOutput=True)
    attn_out = nc.declare_dram_parameter("out_attn", [BL, T], F32, isOutput=True)
    cov_out = nc.declare_dram_parameter("out_cov", [BL, T], F32, isOutput=True)

    with tile.TileContext(nc) as tc:
        with (
            tc.tile_pool(name="const", bufs=1) as const,
            tc.tile_pool(name="ht", bufs=3) as htp,
            tc.tile_pool(name="att", bufs=3) as attp,
            tc.tile_pool(name="rows", bufs=2) as rowp,
            tc.tile_pool(name="rows1", bufs=1) as rowp1,
            tc.tile_pool(name="bc", bufs=BL) as bcp,
            tc.tile_pool(name="scr", bufs=2) as scrp,
            tc.tile_pool(name="psA", bufs=2, space="PSUM") as psA,
            tc.tile_pool(name="psS", bufs=1, space="PSUM") as psS,
            tc.tile_pool(name="psB", bufs=1, space="PSUM") as psB,
        ):
            # ---- constants (issue order matters: prologue inputs first) ----
            wh = const.tile([P, KC, N], BF16)  # [n%128, n//128, m]
            vcol = const.tile([P, KC], BF16)
            ct_all = const.tile([P, BL, KC], F32)  # c_t[p + 128*kc] of batch b
            wcT = const.tile([P, KC], F32)  # W_c[mt*128+p] per-partition scalars
            covb_sb = const.tile([1, BL, T], BF16)  # cov rows (bf16)
            nc.sync.dma_start(out=covb_sb[:], in_=covb_h[:].unsqueeze(0))
            dec_feaT = const.tile([P, MT, BL], F32)  # dec_fea[m, b] bias layout
            ones_col = const.tile([1, P], BF16)  # lhsT for partition broadcast
            nc.any.memset(ones_col[:], 1.0)

            cov_bc_all = []

            # ---- prologue: dec_fea = s_t_hat @ dec_W.T + dec_b  -> [b, m] ----
            with tc.tile_pool(name="prol", bufs=2) as prol:
                st = prol.tile([P, KC, BL], BF16, tag="st")
                nc.sync.dma_start(
                    out=st[:], in_=sT_h[:].rearrange("(kc p) b -> p kc b", p=P)
                )
                ones1 = prol.tile([1, BL], BF16, tag="ones1")
                nc.any.memset(ones1[:], 1.0)
                db = prol.tile([1, N], BF16, tag="db")
                nc.sync.dma_start(out=db[:], in_=decb_h[:])
                dwt = prol.tile([P, KC, N], BF16, tag="dwt")
                for kc in range(KC):
                    nc.sync.dma_start(
                        out=dwt[:, kc, :], in_=dwT_h[kc * P : (kc + 1) * P, :]
                    )
                for kc in range(KC):
                    nc.sync.dma_start(
                        out=wh[:, kc, :], in_=whT_h[kc * P : (kc + 1) * P, :]
                    )
                nc.sync.dma_start(out=vcol[:], in_=vcol_h[:])
                nc.sync.dma_start(out=wcT[:], in_=wcT_h[:])
                # dec_feaT[m, b] = sum_n dec_W[m, n] s_t_hat[b, n] + dec_b[m]
                for mt in range(MT):
                    msl = slice(mt * P, (mt + 1) * P)
                    ps_d = psA.tile([P, BL], F32, tag="psA")
                    for kc in range(KC):
                        nc.tensor.matmul(
                            ps_d[:, :],
                            dwt[:, kc, msl],
                            st[:, kc, :],
                            start=(kc == 0),
                            stop=False,
                        )
                    nc.tensor.matmul(
                        ps_d[:, :], db[:, msl], ones1[:],
                        start=False, stop=True,
                    )
                    nc.vector.tensor_copy(dec_feaT[:, mt, :], ps_d[:, :])

                # broadcast every batch's cov row across partitions up front
                for b in range(BL):
                    cb = bcp.tile([P, T], BF16, tag="covbc")
                    ps_cb = psA.tile([P, T], F32, tag="psA")
                    for th in range(2):
                        sl = slice(th * 512, (th + 1) * 512)
                        nc.tensor.matmul(
                            ps_cb[:, sl], ones_col[:], covb_sb[:, b, sl],
                            start=True, stop=True,
                        )
                    nc.vector.tensor_copy(cb[:], ps_cb[:])
                    cov_bc_all.append(cb)

            # ---- main loop over local batches ----
            def load_ht(b):
                t = htp.tile([P, KC, T], BF16, tag="ht")
                for kc in range(KC):
                    nc.sync.dma_start(
                        out=t[:, kc, :], in_=hT_h[b, kc * P : (kc + 1) * P, :]
                    )
                return t

            def load_rows(b):
                mrow = rowp.tile([1, T], F32, tag="mask")
                nc.sync.dma_start(out=mrow[:], in_=mask_h[b : b + 1, :])
                covrow = rowp.tile([1, T], F32, tag="covrow")
                nc.sync.dma_start(out=covrow[:], in_=cov_h[b : b + 1, :])
                return mrow, covrow

            # pass-B work is deferred and trickled into the next batch's
            # matmul loop so the DVE never bursts >1 op between PSUM
            # evictions (which would stall the PE on PSUM slot reuse).
            pending_pass_b = []

            def issue_pass_b_one():
                if pending_pass_b:
                    pending_pass_b.pop(0)()

            ht_next = load_ht(0)
            rows_next = load_rows(0)
            for b in range(BL):
                ht = ht_next
                mrow, covrow = rows_next

                cov_bc = cov_bc_all[b]
                ps_sc = psS.tile([1, T], F32, tag="psS")
                for mt in range(MT):
                    msl = slice(mt * P, (mt + 1) * P)
                    ps_att = psA.tile([P, T], F32, tag="psA")
                    for th in range(2):
                        sl = slice(th * 512, (th + 1) * 512)
                        for kc in range(KC):
                            nc.tensor.matmul(
                                ps_att[:, sl],
                                wh[:, kc, msl],
                                ht[:, kc, sl],
                                start=(kc == 0),
                                stop=(kc == KC - 1),
                            )
                    # att += W_c[m] * cov[t]  (fused on DVE, in place on PSUM)
                    nc.vector.scalar_tensor_tensor(
                        out=ps_att[:, :], in0=cov_bc[:, :],
                        scalar=wcT[:, mt : mt + 1], in1=ps_att[:, :],
                        op0=ALU.mult, op1=ALU.add,
                    )
                    att = attp.tile([P, T], BF16, tag="att")
                    # att = tanh(psum + dec_fea[m])  (bias folds the dec term)
                    nc.scalar.activation(
                        att[:], ps_att[:], AF.Tanh,
                        bias=dec_feaT[:, mt, b : b + 1],
                    )
                    for th in range(2):
                        sl = slice(th * 512, (th + 1) * 512)
                        nc.tensor.matmul(
                            ps_sc[:, sl],
                            vcol[:, mt : mt + 1],
                            att[:, sl],
                            start=(mt == 0),
                            stop=(mt == MT - 1),
                        )
                    issue_pass_b_one()

                # prefetch next batch while this batch's softmax runs
                if b + 1 < BL:
                    ht_next = load_ht(b + 1)
                    rows_next = load_rows(b + 1)

                # softmax over t (no max-subtraction: |score| <= ||v||_1 ~ 26)
                erow = rowp1.tile([1, T], F32, tag="erow")
                nc.scalar.activation(erow[:], ps_sc[:], AF.Exp)
                emrow = rowp1.tile([1, T], F32, tag="emrow")
                ssum = rowp1.tile([1, 1], F32, tag="ssum")
                nc.vector.scalar_tensor_tensor(
                    out=emrow[:], in0=erow[:], scalar=1.0, in1=mrow[:],
                    op0=ALU.bypass, op1=ALU.mult, accum_out=ssum[:],
                )
                rinv = rowp1.tile([1, 1], F32, tag="rinv")
                nc.vector.reciprocal(rinv[:], ssum[:])
                arow = rowp.tile([1, T], F32, tag="arow")
                nc.vector.tensor_scalar_mul(arow[:], emrow[:], rinv[:])
                nc.sync.dma_start(out=attn_out[b : b + 1, :], in_=arow[:])
                cnrow = rowp1.tile([1, T], F32, tag="cnrow")
                nc.vector.tensor_add(cnrow[:], arow[:], covrow[:])
                nc.sync.dma_start(out=cov_out[b : b + 1, :], in_=cnrow[:])

                # pass B: c_t[n] = sum_t attn[t] * hT[n, t]
                abrow = rowp.tile([1, T], BF16, tag="abrow")
                nc.vector.tensor_copy(abrow[:], arow[:])
                abc = bcp.tile([P, T], BF16, tag="abc")

                def make_pass_b(ht_=ht, abc_=abc, b_=b, abrow_=abrow):
                    def bcast():
                        ps_bc = psB.tile([P, T], F32, tag="psB")
                        for th in range(2):
                            sl = slice(th * 512, (th + 1) * 512)
                            nc.tensor.matmul(
                                ps_bc[:, sl], ones_col[:], abrow_[:, sl],
                                start=True, stop=True,
                            )
                        nc.vector.tensor_copy(abc_[:], ps_bc[:])

                    def stt_one(kc):
                        def run():
                            sc = scrp.tile([P, T], BF16, tag="scr")
                            nc.vector.scalar_tensor_tensor(
                                out=sc[:], in0=ht_[:, kc, :], scalar=1.0,
                                in1=abc_[:], op0=ALU.bypass, op1=ALU.mult,
                                accum_out=ct_all[:, b_, kc : kc + 1],
                            )
                        return run

                    def ct_dma():
                        # c_t[b] out: dest viewed [p, kc] (4B-strided, tiny)
                        nc.sync.dma_start(
                            out=ct_out[b_ : b_ + 1, :].rearrange(
                                "o (k p) -> (o p) k", p=P
                            ),
                            in_=ct_all[:, b_, :],
                        )

                    return (
                        [bcast]
                        + [stt_one(kc) for kc in range(KC)]
                        + [ct_dma]
                    )

                pending_pass_b.extend(make_pass_b())
                issue_pass_b_one()
                issue_pass_b_one()

            while pending_pass_b:
                issue_pass_b_one()

    _legalize_waits(nc)
    return nc


# Walrus rejects instructions whose sync-wait count exceeds the per-opcode
# descriptor slots ("Too many sync wait commands").  Tile can emit 2-3 waits
# on matmuls/DMAs at cross-engine convergence points.  Hoist surplus waits
# onto standalone InstEventSemaphore carriers inserted just before the
# offender in the same engine stream: the engine stalls on the carrier(s),
# then issues the real instruction with a single wait.  Engine streams are
# in-order, so this is semantics-preserving.
_WAIT_SKIP_OPS = {"InstEventSemaphore"}


def _legalize_waits(nc, limit=1):
    import bass_rust

    def make_carrier(engine, wait):
        return mybir.InstNoOp(
            name=nc.get_next_instruction_name(),
            text_hint="waitfix",
            bass_nofuse=True,
            engine=engine,
            sync_info=mybir.SyncInfo(on_wait=[wait], on_update=[]),
        )

    for fn in nc.m.functions:
        for blk in fn.blocks:
            il = blk.instructions
            i = 0
            while i < len(il):
                inst = il[i]
                op = type(inst).__name__
                si = getattr(inst, "sync_info", None)
                if (
                    op in _WAIT_SKIP_OPS
                    or si is None
                    or len(si.on_wait) <= limit
                ):
                    i += 1
                    continue
                waits = list(si.on_wait)
                keep, surplus = waits[-limit:], waits[:-limit]
                carriers = [make_carrier(inst.engine, w) for w in surplus]
                inst.sync_info = bass_rust.SyncInfo(
                    on_wait=keep, on_update=si.on_update
                )
                for k, ev in enumerate(carriers):
                    il.insert(i + k, ev)
                i += len(carriers) + 1


def _get_nc():
    global _NC_CACHE
    if _NC_CACHE is None:
        _NC_CACHE = build_bass()
    return _NC_CACHE


def kernel(s_t_hat, h, enc_padding_mask, coverage, W_h, W_c, dec_W, dec_b, v):
    global LAST_EXEC_NS
    import ml_dtypes

    bf16 = ml_dtypes.bfloat16
    s_t_hat = np.asarray(s_t_hat, dtype=np.float32)
    h = np.asarray(h, dtype=np.float32)
    enc_padding_mask = np.ascontiguousarray(
        np.asarray(enc_padding_mask, dtype=np.float32)
    )
    coverage = np.ascontiguousarray(np.asarray(coverage, dtype=np.float32))
    W_h = np.asarray(W_h, dtype=np.float32)
    W_c = np.asarray(W_c, dtype=np.float32).reshape(1, N)
    dec_W = np.asarray(dec_W, dtype=np.float32)
    dec_b = np.asarray(dec_b, dtype=np.float32).reshape(1, N)
    v = np.asarray(v, dtype=np.float32)

    hT = np.ascontiguousarray(np.transpose(h, (0, 2, 1)).astype(bf16))  # [B, N, T]
    WhT = np.ascontiguousarray(W_h.T.astype(bf16))  # [n, m]
    decWT = np.ascontiguousarray(dec_W.T.astype(bf16))  # [n, m]
    sT = np.ascontiguousarray(s_t_hat.T.astype(bf16))  # [n, B]
    vcol = np.ascontiguousarray(v.reshape(KC, P).T.astype(bf16))  # [p, kc]
    covb = coverage.astype(bf16)
    wcT = np.ascontiguousarray(
        W_c.reshape(KC, P).T.astype(np.float32)
    )  # [p, kc]
    decb_b = np.ascontiguousarray(dec_b.astype(bf16))

    in_maps = []
    for c in range(NCORES):
        bs = slice(c * BL, (c + 1) * BL)
        in_maps.append(
            {
                "hT": hT[bs],
                "cov": coverage[bs],
                "covb": covb[bs],
                "mask": enc_padding_mask[bs],
                "sT": np.ascontiguousarray(sT[:, bs]),
                "WhT": WhT,
                "decWT": decWT,
                "decb": decb_b,
                "WcT": wcT,
                "vcol": vcol,
            }
        )

    nc = _get_nc()
    trace = os.environ.get("BASS_KERNEL_TRACE", "0") == "1"
    res = run_bass_kernel_spmd(
        nc, in_maps, core_ids=list(range(NCORES)), trace=trace
    )
    LAST_EXEC_NS = res.exec_time_ns

    c_t = np.concatenate([res.results[c]["out_ct"] for c in range(NCORES)], axis=0)
    attn = np.concatenate(
        [res.results[c]["out_attn"] for c in range(NCORES)], axis=0
    )
    cov_new = np.concatenate(
        [res.results[c]["out_cov"] for c in range(NCORES)], axis=0
    )
    return (c_t, attn, cov_new)


# revision 44
# speedup vs baseline: 1.4921x; 1.0268x over previous
"""Pointer-generator attention kernel for 8 TRN2 NeuronCores.

Computation (per batch b):
    enc_feat = h[b] @ W_h.T                       # [T, N]
    att      = enc_feat + dec_fea[b] + cov[b,:,None] * W_c
    scores   = tanh(att) @ v                      # [T]
    attn     = exp(scores) * mask / sum(...)      # [T]
    c_t      = attn @ h[b]                        # [N]
    cov_new  = cov + attn

Sharding: data-parallel over batch, 8 batches per core, no collectives.

Device-side layout (per core):
    hT [8, N, T] in bf16 (cast on host) -- h transposed per batch, so the
    contraction dim n sits on SBUF partitions for the main matmul AND the
    t axis is the free dim for the pass-B reduce.  h is read from HBM once.
    att tiles [m=128, t=1024]: lhsT = W_hT chunk (stationary), rhs = hT.
    The rank-1 terms (dec_fea[m] x 1 + W_c[m] x cov[t]) are folded into the
    same PSUM accumulation group as one extra K=2 matmul.
    tanh on ScalarE (PSUM->SBUF eviction), v-dot as M=1 matmuls on PE,
    softmax on single-partition rows (exp has no overflow risk:
    |score| <= ||v||_1 ~ 26), pass B as fused multiply+reduce on VectorE
    over the resident hT tiles.  Matmuls in bf16, accumulation in fp32.
"""

import os
import sys

import numpy as np

sys.path.insert(0, "/opt/trn_rl_repo")

import concourse.bass as bass  # noqa: E402
import concourse.tile as tile  # noqa: E402
from concourse import mybir  # noqa: E402
from concourse.bass_utils import run_bass_kernel_spmd  # noqa: E402

B, T, N = 64, 1024, 1024
NCORES = 8
BL = B // NCORES  # 8 local batches per core
P = 128
KC = N // P  # 8 contraction chunks
MT = N // P  # 8 output row tiles
F32 = mybir.dt.float32
BF16 = mybir.dt.bfloat16
AF = mybir.ActivationFunctionType
ALU = mybir.AluOpType

LAST_EXEC_NS = None
_NC_CACHE = None


def build_bass():
    nc = bass.Bass()

    hT_h = nc.declare_dram_parameter("hT", [BL, N, T], BF16, isOutput=False)
    cov_h = nc.declare_dram_parameter("cov", [BL, T], F32, isOutput=False)
    covb_h = nc.declare_dram_parameter("covb", [BL, T], BF16, isOutput=False)
    mask_h = nc.declare_dram_parameter("mask", [BL, T], F32, isOutput=False)
    sT_h = nc.declare_dram_parameter("sT", [N, BL], BF16, isOutput=False)
    whT_h = nc.declare_dram_parameter("WhT", [N, N], BF16, isOutput=False)
    dwT_h = nc.declare_dram_parameter("decWT", [N, N], BF16, isOutput=False)
    decb_h = nc.declare_dram_parameter("decb", [1, N], BF16, isOutput=False)
    wcT_h = nc.declare_dram_parameter("WcT", [P, KC], F32, isOutput=False)
    vcol_h = nc.declare_dram_parameter("vcol", [P, KC], BF16, isOutput=False)

    ct_out = nc.declare_dram_parameter("out_ct", [BL, N], F32, isOutput=True)
    attn_out = nc.declare_dram_parameter("out_attn", [BL, T], F32, isOutput=True)
    cov_out = nc.declare_dram_parameter("out_cov", [BL, T], F32, isOutput=True)

    with tile.TileContext(nc) as tc:
        with (
            tc.tile_pool(name="const", bufs=1) as const,
            tc.tile_pool(name="ht", bufs=3) as htp,
            tc.tile_pool(name="att", bufs=3) as attp,
            tc.tile_pool(name="rows", bufs=2) as rowp,
            tc.tile_pool(name="rows1", bufs=1) as rowp1,
            tc.tile_pool(name="bc", bufs=BL) as bcp,
            tc.tile_pool(name="scr", bufs=2) as scrp,
            tc.tile_pool(name="psA", bufs=2, space="PSUM") as psA,
            tc.tile_pool(name="psS", bufs=1, space="PSUM") as psS,
            tc.tile_pool(name="psB", bufs=1, space="PSUM") as psB,
        ):
            # ---- PE warm-up: dummy matmuls while the first DMAs land, so
            # the HAM clock gate reaches 2.4 GHz before real work starts ----
            ones_col = const.tile([1, P], BF16)  # also lhsT for broadcasts
            nc.any.memset(ones_col[:], 1.0)
            warm_row = const.tile([1, 512], BF16)
            nc.any.memset(warm_row[:], 0.0)
            ps_w = psA.tile([P, T], F32, tag="psA")
            for _ in range(24):
                nc.tensor.matmul(
                    ps_w[:, 0:512], ones_col[:], warm_row[:],
                    start=True, stop=True,
                )

            # ---- constants (issue order matters: prologue inputs first) ----
            wh = const.tile([P, KC, N], BF16)  # [n%128, n//128, m]
            vcol = const.tile([P, KC], BF16)
            ct_all = const.tile([P, BL, KC], F32)  # c_t[p + 128*kc] of batch b
            wcT = const.tile([P, KC], F32)  # W_c[mt*128+p] per-partition scalars
            covb_sb = const.tile([1, BL, T], BF16)  # cov rows (bf16)
            nc.sync.dma_start(out=covb_sb[:], in_=covb_h[:].unsqueeze(0))
            dec_feaT = const.tile([P, MT, BL], F32)  # dec_fea[m, b] bias layout

            cov_bc_all = []

            # ---- prologue: dec_fea = s_t_hat @ dec_W.T + dec_b  -> [b, m] ----
            with tc.tile_pool(name="prol", bufs=2) as prol:
                st = prol.tile([P, KC, BL], BF16, tag="st")
                nc.sync.dma_start(
                    out=st[:], in_=sT_h[:].rearrange("(kc p) b -> p kc b", p=P)
                )
                ones1 = prol.tile([1, BL], BF16, tag="ones1")
                nc.any.memset(ones1[:], 1.0)
                db = prol.tile([1, N], BF16, tag="db")
                nc.sync.dma_start(out=db[:], in_=decb_h[:])
                dwt = prol.tile([P, KC, N], BF16, tag="dwt")
                for kc in range(KC):
                    nc.sync.dma_start(
                        out=dwt[:, kc, :], in_=dwT_h[kc * P : (kc + 1) * P, :]
                    )
                for kc in range(KC):
                    nc.sync.dma_start(
                        out=wh[:, kc, :], in_=whT_h[kc * P : (kc + 1) * P, :]
                    )
                nc.sync.dma_start(out=vcol[:], in_=vcol_h[:])
                nc.sync.dma_start(out=wcT[:], in_=wcT_h[:])
                # dec_feaT[m, b] = sum_n dec_W[m, n] s_t_hat[b, n] + dec_b[m]
                for mt in range(MT):
                    msl = slice(mt * P, (mt + 1) * P)
                    ps_d = psA.tile([P, BL], F32, tag="psA")
                    for kc in range(KC):
                        nc.tensor.matmul(
                            ps_d[:, :],
                            dwt[:, kc, msl],
                            st[:, kc, :],
                            start=(kc == 0),
                            stop=False,
                        )
                    nc.tensor.matmul(
                        ps_d[:, :], db[:, msl], ones1[:],
                        start=False, stop=True,
                    )
                    nc.vector.tensor_copy(dec_feaT[:, mt, :], ps_d[:, :])

                # broadcast every batch's cov row across partitions up front
                for b in range(BL):
                    cb = bcp.tile([P, T], BF16, tag="covbc")
                    ps_cb = psA.tile([P, T], F32, tag="psA")
                    for th in range(2):
                        sl = slice(th * 512, (th + 1) * 512)
                        nc.tensor.matmul(
                            ps_cb[:, sl], ones_col[:], covb_sb[:, b, sl],
                            start=True, stop=True,
                        )
                    nc.vector.tensor_copy(cb[:], ps_cb[:])
                    cov_bc_all.append(cb)

            # ---- main loop over local batches ----
            def load_ht(b):
                t = htp.tile([P, KC, T], BF16, tag="ht")
                for kc in range(KC):
                    nc.sync.dma_start(
                        out=t[:, kc, :], in_=hT_h[b, kc * P : (kc + 1) * P, :]
                    )
                return t

            def load_rows(b):
                mrow = rowp.tile([1, T], F32, tag="mask")
                nc.sync.dma_start(out=mrow[:], in_=mask_h[b : b + 1, :])
                covrow = rowp.tile([1, T], F32, tag="covrow")
                nc.sync.dma_start(out=covrow[:], in_=cov_h[b : b + 1, :])
                return mrow, covrow

            # pass-B work is deferred and trickled into the next batch's
            # matmul loop so the DVE never bursts >1 op between PSUM
            # evictions (which would stall the PE on PSUM slot reuse).
            pending_pass_b = []

            def issue_pass_b_one():
                if pending_pass_b:
                    pending_pass_b.pop(0)()

            ht_next = load_ht(0)
            rows_next = load_rows(0)
            for b in range(BL):
                ht = ht_next
                mrow, covrow = rows_next

                cov_bc = cov_bc_all[b]
                ps_sc = psS.tile([1, T], F32, tag="psS")
                for mt in range(MT):
                    msl = slice(mt * P, (mt + 1) * P)
                    ps_att = psA.tile([P, T], F32, tag="psA")
                    for th in range(2):
                        sl = slice(th * 512, (th + 1) * 512)
                        for kc in range(KC):
                            nc.tensor.matmul(
                                ps_att[:, sl],
                                wh[:, kc, msl],
                                ht[:, kc, sl],
                                start=(kc == 0),
                                stop=(kc == KC - 1),
                            )
                    # att += W_c[m] * cov[t]  (fused on DVE, in place on PSUM)
                    nc.vector.scalar_tensor_tensor(
                        out=ps_att[:, :], in0=cov_bc[:, :],
                        scalar=wcT[:, mt : mt + 1], in1=ps_att[:, :],
                        op0=ALU.mult, op1=ALU.add,
                    )
                    att = attp.tile([P, T], BF16, tag="att")
                    # att = tanh(psum + dec_fea[m])  (bias folds the dec term)
                    nc.scalar.activation(
                        att[:], ps_att[:], AF.Tanh,
                        bias=dec_feaT[:, mt, b : b + 1],
                    )
                    for th in range(2):
                        sl = slice(th * 512, (th + 1) * 512)
                        nc.tensor.matmul(
                            ps_sc[:, sl],
                            vcol[:, mt : mt + 1],
                            att[:, sl],
                            start=(mt == 0),
                            stop=(mt == MT - 1),
                        )
                    issue_pass_b_one()

                # prefetch next batch while this batch's softmax runs
                if b + 1 < BL:
                    ht_next = load_ht(b + 1)
                    rows_next = load_rows(b + 1)

                # softmax over t (no max-subtraction: |score| <= ||v||_1 ~ 26)
                erow = rowp1.tile([1, T], F32, tag="erow")
                nc.scalar.activation(erow[:], ps_sc[:], AF.Exp)
                emrow = rowp1.tile([1, T], F32, tag="emrow")
                ssum = rowp1.tile([1, 1], F32, tag="ssum")
                nc.vector.scalar_tensor_tensor(
                    out=emrow[:], in0=erow[:], scalar=1.0, in1=mrow[:],
                    op0=ALU.bypass, op1=ALU.mult, accum_out=ssum[:],
                )
                rinv = rowp1.tile([1, 1], F32, tag="rinv")
                nc.vector.reciprocal(rinv[:], ssum[:])
                arow = rowp.tile([1, T], F32, tag="arow")
                nc.vector.tensor_scalar_mul(arow[:], emrow[:], rinv[:])
                nc.sync.dma_start(out=attn_out[b : b + 1, :], in_=arow[:])
                cnrow = rowp1.tile([1, T], F32, tag="cnrow")
                nc.vector.tensor_add(cnrow[:], arow[:], covrow[:])
                nc.sync.dma_start(out=cov_out[b : b + 1, :], in_=cnrow[:])

                # pass B: c_t[n] = sum_t attn[t] * hT[n, t]
                abrow = rowp.tile([1, T], BF16, tag="abrow")
                nc.vector.tensor_copy(abrow[:], arow[:])

                def make_pass_b(ht_=ht, b_=b, abrow_=abrow):
                    ps_box = []

                    def bcast():
                        ps_bc = psB.tile([P, T], F32, tag="psB")
                        ps_box.append(ps_bc)
                        for th in range(2):
                            sl = slice(th * 512, (th + 1) * 512)
                            nc.tensor.matmul(
                                ps_bc[:, sl], ones_col[:], abrow_[:, sl],
                                start=True, stop=True,
                            )

                    def stt_one(kc):
                        def run():
                            sc = scrp.tile([P, T], BF16, tag="scr")
                            nc.vector.scalar_tensor_tensor(
                                out=sc[:], in0=ht_[:, kc, :], scalar=1.0,
                                in1=ps_box[0][:], op0=ALU.bypass, op1=ALU.mult,
                                accum_out=ct_all[:, b_, kc : kc + 1],
                            )
                        return run

                    def ct_dma():
                        # c_t[b] out: dest viewed [p, kc] (4B-strided, tiny)
                        nc.sync.dma_start(
                            out=ct_out[b_ : b_ + 1, :].rearrange(
                                "o (k p) -> (o p) k", p=P
                            ),
                            in_=ct_all[:, b_, :],
                        )

                    return (
                        [bcast]
                        + [stt_one(kc) for kc in range(KC)]
                        + [ct_dma]
                    )

                pending_pass_b.extend(make_pass_b())
                issue_pass_b_one()
                issue_pass_b_one()

            while pending_pass_b:
                issue_pass_b_one()

    _legalize_waits(nc)
    return nc


# Walrus rejects instructions whose sync-wait count exceeds the per-opcode
# descriptor slots ("Too many sync wait commands").  Tile can emit 2-3 waits
# on matmuls/DMAs at cross-engine convergence points.  Hoist surplus waits
# onto standalone InstEventSemaphore carriers inserted just before the
# offender in the same engine stream: the engine stalls on the carrier(s),
# then issues the real instruction with a single wait.  Engine streams are
# in-order, so this is semantics-preserving.
_WAIT_SKIP_OPS = {"InstEventSemaphore"}


def _legalize_waits(nc, limit=1):
    import bass_rust

    def make_carrier(engine, wait):
        return mybir.InstNoOp(
            name=nc.get_next_instruction_name(),
            text_hint="waitfix",
            bass_nofuse=True,
            engine=engine,
            sync_info=mybir.SyncInfo(on_wait=[wait], on_update=[]),
        )

    for fn in nc.m.functions:
        for blk in fn.blocks:
            il = blk.instructions
            i = 0
            while i < len(il):
                inst = il[i]
                op = type(inst).__name__
                si = getattr(inst, "sync_info", None)
                if (
                    op in _WAIT_SKIP_OPS
                    or si is None
                    or len(si.on_wait) <= limit
                ):
                    i += 1
                    continue
                waits = list(si.on_wait)
                keep, surplus = waits[-limit:], waits[:-limit]
                carriers = [make_carrier(inst.engine, w) for w in surplus]
                inst.sync_info = bass_rust.SyncInfo(
                    on_wait=keep, on_update=si.on_update
                )
                for k, ev in enumerate(carriers):
                    il.insert(i + k, ev)
                i += len(carriers) + 1


def _get_nc():
    global _NC_CACHE
    if _NC_CACHE is None:
        _NC_CACHE = build_bass()
    return _NC_CACHE


def kernel(s_t_hat, h, enc_padding_mask, coverage, W_h, W_c, dec_W, dec_b, v):
    global LAST_EXEC_NS
    import ml_dtypes

    bf16 = ml_dtypes.bfloat16
    s_t_hat = np.asarray(s_t_hat, dtype=np.float32)
    h = np.asarray(h, dtype=np.float32)
    enc_padding_mask = np.ascontiguousarray(
        np.asarray(enc_padding_mask, dtype=np.float32)
    )
    coverage = np.ascontiguousarray(np.asarray(coverage, dtype=np.float32))
    W_h = np.asarray(W_h, dtype=np.float32)
    W_c = np.asarray(W_c, dtype=np.float32).reshape(1, N)
    dec_W = np.asarray(dec_W, dtype=np.float32)
    dec_b = np.asarray(dec_b, dtype=np.float32).reshape(1, N)
    v = np.asarray(v, dtype=np.float32)

    hT = np.ascontiguousarray(np.transpose(h, (0, 2, 1)).astype(bf16))  # [B, N, T]
    WhT = np.ascontiguousarray(W_h.T.astype(bf16))  # [n, m]
    decWT = np.ascontiguousarray(dec_W.T.astype(bf16))  # [n, m]
    sT = np.ascontiguousarray(s_t_hat.T.astype(bf16))  # [n, B]
    vcol = np.ascontiguousarray(v.reshape(KC, P).T.astype(bf16))  # [p, kc]
    covb = coverage.astype(bf16)
    wcT = np.ascontiguousarray(
        W_c.reshape(KC, P).T.astype(np.float32)
    )  # [p, kc]
    decb_b = np.ascontiguousarray(dec_b.astype(bf16))

    in_maps = []
    for c in range(NCORES):
        bs = slice(c * BL, (c + 1) * BL)
        in_maps.append(
            {
                "hT": hT[bs],
                "cov": coverage[bs],
                "covb": covb[bs],
                "mask": enc_padding_mask[bs],
                "sT": np.ascontiguousarray(sT[:, bs]),
                "WhT": WhT,
                "decWT": decWT,
                "decb": decb_b,
                "WcT": wcT,
                "vcol": vcol,
            }
        )

    nc = _get_nc()
    trace = os.environ.get("BASS_KERNEL_TRACE", "0") == "1"
    res = run_bass_kernel_spmd(
        nc, in_maps, core_ids=list(range(NCORES)), trace=trace
    )
    LAST_EXEC_NS = res.exec_time_ns

    c_t = np.concatenate([res.results[c]["out_ct"] for c in range(NCORES)], axis=0)
    attn = np.concatenate(
        [res.results[c]["out_attn"] for c in range(NCORES)], axis=0
    )
    cov_new = np.concatenate(
        [res.results[c]["out_cov"] for c in range(NCORES)], axis=0
    )
    return (c_t, attn, cov_new)
